# revision 1
# baseline (speedup 1.0000x reference)
"""Trainium2 Bass kernel for nn_AttentionLayer (B=2048, N=64, D=256, H=16).

Math (per batch row b):
  total = sum_n m[b,n,:];  diff = (N*m - total) * item[b]
  logits = relu(diff @ W1 + b1) @ W2;  out = softmax(logits over n)

Restructured:  v = m*item (in d-major layout);  G = v @ W1  (PE, fp32r)
  pre = N*G - sum_n(N*G) + b1 = 64*(G - s_G) + b1   (s_G via DVE segmented
  reduce + rank-16 accumulating correction matmul with stride-0 rhs)
  relu on ACT (scale=64, bias=b1), mm2 on PE, softmax via small DRAM bounce
  back to natural [b, n] layout.

Layout: members loaded naturally [128 bn, d]; transposed on PE (exact fp32
identity transpose-mode matmuls) to [d, bn]; item multiply as one DVE
tensor_tensor per group with a stride-0 broadcast AP over n.

Sharding: pure data-parallel over B across 8 cores (one SPMD NEFF).
"""
import numpy as np
from contextlib import ExitStack

import concourse.bacc as bacc
import concourse.tile as tile
import concourse.mybir as mybir
from concourse import bass_utils

B, N, D, H = 2048, 64, 256, 16
NCORES = 8
BL = B // NCORES          # 256 batch rows per core
GB = 8                    # batch rows per group
NG = BL // GB             # 32 groups
BLK = GB * N // 128       # 4 128-row bn blocks per group
CH = D // 128             # 2 d chunks
FREE = GB * N             # 512 bn columns per group

f32 = mybir.dt.float32
f32r = mybir.dt.float32r
AF = mybir.ActivationFunctionType
OP = mybir.AluOpType

_cache = {}


def _build(repeat=1, small=False):
    nc = bacc.Bacc("TRN2", target_bir_lowering=False)
    mem_rows = FREE if small else BL * N
    mem = nc.dram_tensor("mem", [mem_rows, D], f32r, kind="ExternalInput")
    itT = nc.dram_tensor("itT", [CH, 128, BL], f32, kind="ExternalInput")
    W1d = nc.dram_tensor("W1d", [D, H], f32r, kind="ExternalInput")
    b1d = nc.dram_tensor("b1d", [H, 1], f32, kind="ExternalInput")
    W2d = nc.dram_tensor("W2d", [H, 1], f32r, kind="ExternalInput")
    idd = nc.dram_tensor("idd", [128, 128], f32r, kind="ExternalInput")
    nId = nc.dram_tensor("nId", [H, H], f32r, kind="ExternalInput")
    out = nc.dram_tensor("out", [BL, N], f32, kind="ExternalOutput")
    lgs = nc.dram_tensor("lgs", [BL, N], f32, kind="Internal")

    with tile.TileContext(nc) as tc, ExitStack() as ctx:
        const = ctx.enter_context(tc.tile_pool(name="const", bufs=1))
        mpool = ctx.enter_context(tc.tile_pool(name="mpool", bufs=4))
        spool = ctx.enter_context(tc.tile_pool(name="spool", bufs=3))
        hpool = ctx.enter_context(tc.tile_pool(name="hpool", bufs=3))
        tps = ctx.enter_context(tc.tile_pool(name="tps", bufs=2, space="PSUM"))
        gps = ctx.enter_context(tc.tile_pool(name="gps", bufs=2, space="PSUM"))
        lps = ctx.enter_context(tc.tile_pool(name="lps", bufs=2, space="PSUM"))

        ident = const.tile([128, 128], f32r)
        itT_sb = const.tile([128, CH, BL], f32)
        W1_sb = const.tile([128, CH, H], f32r)
        b1_sb = const.tile([H, 1], f32)
        W2_sb = const.tile([H, 1], f32r)
        nI_sb = const.tile([H, H], f32r)
        nc.gpsimd.dma_start(out=ident, in_=idd[:, :])
        nc.gpsimd.dma_start(
            out=itT_sb, in_=itT[:, :, :].rearrange("c p b -> p c b"))
        nc.gpsimd.dma_start(
            out=W1_sb, in_=W1d[:, :].rearrange("(c p) h -> p c h", p=128))
        nc.gpsimd.dma_start(out=b1_sb, in_=b1d[:, :])
        nc.gpsimd.dma_start(out=W2_sb, in_=W2d[:, :])
        nc.gpsimd.dma_start(out=nI_sb, in_=nId[:, :])

        import itertools
        lsb_all = const.tile([1, NG, GB, N], f32)
        for rep, g in itertools.product(range(repeat), range(NG)):
            # natural-layout load: [128 bn | blk | chunk | d]
            m_g = mpool.tile([128, BLK, CH, 128], f32r)
            g_src = 0 if small else g
            src = mem[g_src * FREE:(g_src + 1) * FREE, :].rearrange(
                "(blk p) (c d) -> p blk c d", p=128, c=CH)
            nc.sync.dma_start(out=m_g, in_=src)

            # PE transpose to [d | chunk-major columns of bn]
            T = tps.tile([128, CH, GB, N], f32r)
            for c in range(CH):
                for i in range(BLK):
                    nc.tensor.transpose(
                        T[:, c, i * 2:(i + 1) * 2, :],
                        m_g[:, i, c, :], ident[:])

            # item scale: one DVE TT, stride-0 broadcast over n; round to f32r
            scaled = spool.tile([128, CH, GB, N], f32r)
            it_b = itT_sb[:, :, g * GB:(g + 1) * GB].unsqueeze(3).broadcast_to(
                [128, CH, GB, N])
            nc.vector.tensor_mul(scaled[:], T[:].bitcast(f32), it_b)

            # mm1: G[h, bn] accumulated over 2 d-chunks
            G = gps.tile([H, GB, N], f32)
            for c in range(CH):
                nc.tensor.matmul(
                    G[:, :, :], W1_sb[:, c, :], scaled[:, c, :, :],
                    start=(c == 0), stop=False)

            # s_G[h, b] = sum_n G; correction matmul adds -s_G broadcast
            sG = hpool.tile([H, GB], f32r, tag="sG")
            with nc.allow_low_precision(reason="f32r out, fp32 accum internal"):
                nc.vector.reduce_sum(sG[:], G[:, :, :],
                                     axis=mybir.AxisListType.X)
            nc.tensor.matmul(
                G[:, :, :], nI_sb[:],
                sG[:].unsqueeze(2).broadcast_to([H, GB, N]),
                start=False, stop=True)

            # relu(64*(G - s_G) + b1) on ACT, rounded to f32r for mm2
            hrel = hpool.tile([H, GB, N], f32r, tag="hrel")
            nc.scalar.activation(hrel[:], G[:, :, :], AF.Relu,
                                 bias=b1_sb[:], scale=64.0)

            # mm2 -> logits [1, 512]
            L = lps.tile([1, GB, N], f32)
            nc.tensor.matmul(L[:, :, :], W2_sb[:], hrel[:], start=True,
                             stop=True)
            nc.scalar.copy(lsb_all[:, g, :, :], L[:, :, :])

        # single logits writeback, then phase 2 softmax in [b, n] layout
        nc.sync.dma_start(out=lgs[:, :].unsqueeze(0),
                          in_=lsb_all[:].rearrange("o r b n -> o (r b) n"))
        p2 = ctx.enter_context(tc.tile_pool(name="p2", bufs=2))
        for t in range(BL // 128):
            lg_t = p2.tile([128, N], f32)
            nc.gpsimd.dma_start(out=lg_t, in_=lgs[t * 128:(t + 1) * 128, :])
            nmax = p2.tile([128, 1], f32)
            nc.vector.tensor_reduce(nmax[:], lg_t[:], axis=mybir.AxisListType.X,
                                    op=OP.max, negate=True)
            ex = p2.tile([128, N], f32)
            ssum = p2.tile([128, 1], f32)
            nc.scalar.activation(ex[:], lg_t[:], AF.Exp, bias=nmax[:],
                                 scale=1.0, accum_out=ssum[:])
            rs = p2.tile([128, 1], f32)
            nc.vector.reciprocal(rs[:], ssum[:])
            o_t = p2.tile([128, N], f32)
            nc.vector.tensor_scalar_mul(o_t[:], ex[:], rs[:])
            nc.gpsimd.dma_start(out=out[t * 128:(t + 1) * 128, :], in_=o_t[:])

    nc.compile()
    return nc


def make_in_maps(members_embeds, item_embeds, W1, b1, W2, b2=None):
    members_embeds = np.ascontiguousarray(members_embeds, dtype=np.float32)
    item_embeds = np.ascontiguousarray(item_embeds, dtype=np.float32)
    W1 = np.ascontiguousarray(W1, dtype=np.float32)
    b1 = np.asarray(b1, dtype=np.float32).reshape(H, 1)
    W2 = np.ascontiguousarray(W2, dtype=np.float32).reshape(H, 1)
    # b2 drops out of softmax entirely.

    idv = np.eye(128, dtype=np.float32)
    # correction must subtract s_G, not 64*s_G: relu scale=64 multiplies
    # (G - s_G/64), so the correction weight is -I/64.
    nIv = (-np.eye(H, dtype=np.float32) / 64.0)

    in_maps = []
    for k in range(NCORES):
        mem_k = members_embeds[k * BL:(k + 1) * BL].reshape(BL * N, D)
        it_k = item_embeds[k * BL:(k + 1) * BL]            # [BL, D]
        itT_k = np.ascontiguousarray(
            it_k.T.reshape(CH, 128, BL), dtype=np.float32)
        in_maps.append({
            "mem": np.ascontiguousarray(mem_k),
            "itT": itT_k,
            "W1d": W1, "b1d": b1, "W2d": W2,
            "idd": idv, "nId": nIv,
        })

    return in_maps


def kernel(members_embeds, item_embeds, W1, b1, W2, b2=None):
    if "nc" not in _cache:
        _cache["nc"] = _build()
    nc = _cache["nc"]
    in_maps = make_in_maps(members_embeds, item_embeds, W1, b1, W2, b2)
    res = bass_utils.run_bass_kernel_spmd(
        nc, in_maps, core_ids=list(range(NCORES)))
    return np.concatenate([r["out"] for r in res.results], axis=0)

